# revision 1
# baseline (speedup 1.0000x reference)
"""GAT 2-layer kernel for Trainium2, 8 NeuronCores.

Strategy (per sharding hint): permute + bin-pack nodes into 784 balanced
dst-blocks of 128 slots; 98 blocks per core. Edge aggregation runs as
one-hot-mask matmuls accumulating in PSUM per dst-block. Node features for
each block's incident edges are staged edge-aligned by the host between
launches (halo exchange); all arithmetic (dense matmuls, attention logits,
exp, softmax normalization, relu, aggregation) runs on device.

Launches:
  A : h1x = X @ [W1|As|Ad]           (node-sharded dense matmul)
  B : layer-1 edge aggregation -> g  (dst-sharded)
  A2: h2x = g @ [W2|as2|ad2]         (node-sharded dense matmul)
  C : layer-2 edge aggregation -> out
"""
import os
import heapq
import numpy as np

import concourse.bacc as bacc
import concourse.bass as bass
import concourse.mybir as mybir
import concourse.tile as tile
from concourse import bass_utils

F32 = mybir.dt.float32
I32 = mybir.dt.int32
P = 128
NCORE = 8
NEG = 0.2

_TRACE = bool(int(os.environ.get("GAT_TRACE", "0")))
LAST_EXEC_NS = {}
LAST_WALL = {}
DBG = {}


def _run(nc, in_maps, tag):
    import time as _time
    t0 = _time.time()
    res = bass_utils.run_bass_kernel_spmd(
        nc, in_maps, core_ids=list(range(NCORE)), trace=False)
    LAST_WALL[tag] = _time.time() - t0
    LAST_EXEC_NS[tag] = res.exec_time_ns
    return res.results


# ---------------------------------------------------------------- dense
def _build_dense(K, Nloc, M):
    """out[Nloc, M] = inT[K, Nloc].T @ W[K, M], fp32. K in {128, 256}."""
    nc = bacc.Bacc("TRN2", target_bir_lowering=False, debug=False)
    inT_d = nc.dram_tensor("inT", [K, Nloc], F32, kind="ExternalInput")
    w_d = nc.dram_tensor("w", [K, M], F32, kind="ExternalInput")
    out_d = nc.dram_tensor("out", [Nloc, M], F32, kind="ExternalOutput")
    nk = K // P
    with tile.TileContext(nc) as tc:
        with (
            tc.tile_pool(name="wp", bufs=1) as wp,
            tc.tile_pool(name="xp", bufs=3) as xp,
            tc.tile_pool(name="pp", bufs=2, space="PSUM") as pp,
            tc.tile_pool(name="op", bufs=2) as op,
        ):
            wt = wp.tile([P, nk * M], F32)
            for k in range(nk):
                nc.sync.dma_start(wt[:, k * M:(k + 1) * M], w_d[k * P:(k + 1) * P, :])
            for i in range(Nloc // P):
                xt = xp.tile([P, nk * P], F32, tag="x", name=f"x{i}")
                for k in range(nk):
                    nc.sync.dma_start(
                        xt[:, k * P:(k + 1) * P],
                        inT_d[k * P:(k + 1) * P, i * P:(i + 1) * P])
                ps = pp.tile([P, M], F32, tag="ps", name=f"ps{i}")
                for k in range(nk):
                    nc.tensor.matmul(
                        out=ps[:], lhsT=xt[:, k * P:(k + 1) * P],
                        rhs=wt[:, k * M:(k + 1) * M],
                        start=(k == 0), stop=(k == nk - 1))
                ot = op.tile([P, M], F32, tag="o", name=f"o{i}")
                nc.scalar.activation(out=ot[:], in_=ps[:],
                                     func=mybir.ActivationFunctionType.Copy)
                nc.sync.dma_start(out_d[i * P:(i + 1) * P, :], ot[:])
    nc.compile()
    return nc


# ---------------------------------------------------------------- edge agg
def _build_edge(NB, T, H, C, relu_out, has_bias):
    """One GAT aggregation layer, dst-sharded.

    hsrcE [NB*128, T*H*(C+2)]: per block, edge-aligned gathered rows
        [featH0(C) | 1 | 0 | featH1(C) | 1 | 0] (H=2) or [feat | 1 | 0] (H=1)
    asrcE/adstE [128, NT*H] f32 logit halves; dstloc [128, NT] f32.
    out [NB*128, H*C] f32 = aggregated (normalized, +bias, relu optional).
    """
    CP = C + 2
    G = H * CP
    NT = NB * T
    nc = bacc.Bacc("TRN2", target_bir_lowering=False, debug=False)
    hs_d = nc.dram_tensor("hsrcE", [NB * P, T * G], F32, kind="ExternalInput")
    as_d = nc.dram_tensor("asrcE", [P, NT * H], F32, kind="ExternalInput")
    ad_d = nc.dram_tensor("adstE", [P, NT * H], F32, kind="ExternalInput")
    dl_d = nc.dram_tensor("dstloc", [P, NT], F32, kind="ExternalInput")
    if has_bias:
        b_d = nc.dram_tensor("biasbc", [P, H * C], F32, kind="ExternalInput")
    out_d = nc.dram_tensor("out", [NB * P, H * C], F32, kind="ExternalOutput")

    with tile.TileContext(nc) as tc:
        with (
            tc.tile_pool(name="st", bufs=1) as st,
            tc.tile_pool(name="hp", bufs=3) as hp,
            tc.tile_pool(name="sp", bufs=6) as sp,
            tc.tile_pool(name="pp", bufs=4, space="PSUM") as pp,
            tc.tile_pool(name="ep", bufs=2) as ep,
        ):
            iota_i = st.tile([P, P], I32)
            nc.gpsimd.iota(iota_i[:], pattern=[[1, P]], base=0, channel_multiplier=0)
            iota_f = st.tile([P, P], F32)
            nc.vector.tensor_copy(iota_f[:], iota_i[:])
            dls = st.tile([P, NT], F32)
            nc.sync.dma_start(dls[:], dl_d[:, :])
            if has_bias:
                bb = st.tile([P, H * C], F32)
                nc.sync.dma_start(bb[:], b_d[:, :])
            # ex = exp(lrelu(asrc + adst)); big streaming ops
            exb = st.tile([P, NT * H], F32)
            tas = st.tile([P, NT * H], F32)
            nc.sync.dma_start(exb[:], as_d[:, :])
            nc.sync.dma_start(tas[:], ad_d[:, :])
            CH = 8192
            for o in range(0, NT * H, CH):
                e = min(o + CH, NT * H)
                nc.vector.tensor_tensor(out=tas[:, o:e], in0=tas[:, o:e],
                                        in1=exb[:, o:e], op=mybir.AluOpType.add)
                nc.vector.scalar_tensor_tensor(
                    out=tas[:, o:e], in0=tas[:, o:e], scalar=NEG, in1=tas[:, o:e],
                    op0=mybir.AluOpType.mult, op1=mybir.AluOpType.max)
                nc.scalar.activation(out=exb[:, o:e], in_=tas[:, o:e],
                                     func=mybir.ActivationFunctionType.Exp)

            for b in range(NB):
                hb = hp.tile([P, T * G], F32, tag="h", name=f"h{b}")
                nc.sync.dma_start(hb[:], hs_d[b * P:(b + 1) * P, :])
                pss = []
                for h in range(H):
                    ph = pp.tile([P, CP], F32, tag=f"ps{h}", name=f"ps{b}_{h}")
                    pss.append(ph)
                for t in range(T):
                    nt = b * T + t
                    for h in range(H):
                        S = sp.tile([P, P], F32, tag="S", name=f"S{b}_{t}_{h}")
                        nc.vector.tensor_scalar(
                            out=S[:], in0=iota_f[:],
                            scalar1=dls[:, nt:nt + 1],
                            scalar2=exb[:, nt * H + h:nt * H + h + 1],
                            op0=mybir.AluOpType.is_equal,
                            op1=mybir.AluOpType.mult)
                        nc.tensor.matmul(
                            out=pss[h][:], lhsT=S[:],
                            rhs=hb[:, t * G + h * CP:t * G + (h + 1) * CP],
                            start=(t == 0), stop=(t == T - 1))
                # epilogue
                r = ep.tile([P, H], F32, tag="r", name=f"r{b}")
                for h in range(H):
                    nc.vector.reciprocal(r[:, h:h + 1], pss[h][:, C:C + 1])
                og = ep.tile([P, H * C], F32, tag="og", name=f"og{b}")
                for h in range(H):
                    if has_bias:
                        nc.vector.tensor_scalar(
                            out=og[:, h * C:(h + 1) * C], in0=pss[h][:, 0:C],
                            scalar1=r[:, h:h + 1], scalar2=None,
                            op0=mybir.AluOpType.mult)
                    else:
                        fn = (mybir.ActivationFunctionType.Relu if relu_out
                              else mybir.ActivationFunctionType.Copy)
                        nc.scalar.activation(out=og[:, h * C:(h + 1) * C],
                                             in_=pss[h][:, 0:C], func=fn,
                                             scale=r[:, h:h + 1])
                if has_bias:
                    nc.vector.tensor_tensor(out=og[:], in0=og[:], in1=bb[:],
                                            op=mybir.AluOpType.add)
                    if relu_out:
                        nc.vector.tensor_scalar(
                            out=og[:], in0=og[:], scalar1=0.0, scalar2=None,
                            op0=mybir.AluOpType.max)
                nc.sync.dma_start(out_d[b * P:(b + 1) * P, :], og[:])
    nc.compile()
    return nc


# ---------------------------------------------------------------- host side
def _binpack(deg, nblk):
    """Assign each node to a block (128 slots each), balancing edge load."""
    N = len(deg)
    order = np.argsort(-deg, kind="stable")
    heap = [(0, 0, b) for b in range(nblk)]
    heapq.heapify(heap)
    slot_of_node = np.empty(N, dtype=np.int64)
    counts = np.zeros(nblk, dtype=np.int64)
    for n in order:
        load, cnt, b = heapq.heappop(heap)
        slot_of_node[n] = b * P + cnt
        counts[b] = cnt + 1
        if cnt + 1 < P:
            heapq.heappush(heap, (load + int(deg[n]), cnt + 1, b))
    return slot_of_node, counts


def kernel(X, E, W1, att_src1, att_dst1, b1, W2, att_src2, att_dst2, b2):
    X = np.asarray(X, np.float32)
    E = np.asarray(E)
    N, F = X.shape
    H1, C1 = np.asarray(att_src1).shape
    C2 = np.asarray(att_src2).shape[1]
    NBPC = (N + NCORE * P - 1) // (NCORE * P)      # blocks per core
    NBLK = NBPC * NCORE
    NSLOT = NBLK * P
    NLOC = NBPC * P

    src = np.concatenate([E[0].astype(np.int64), np.arange(N, dtype=np.int64)])
    dst = np.concatenate([E[1].astype(np.int64), np.arange(N, dtype=np.int64)])

    deg = np.bincount(dst, minlength=N) + 0
    slot_of_node, counts = _binpack(deg, NBLK)
    node_of_slot = np.full(NSLOT, -1, dtype=np.int64)
    node_of_slot[slot_of_node] = np.arange(N)

    sslot = slot_of_node[src]
    dslot = slot_of_node[dst]
    # dummy keepalive edges for empty slots
    dummies = np.nonzero(node_of_slot < 0)[0]
    sslot = np.concatenate([sslot, np.zeros(len(dummies), np.int64)])
    dslot = np.concatenate([dslot, dummies])
    keep = np.concatenate([np.zeros(len(src), bool), np.ones(len(dummies), bool)])

    order = np.argsort(dslot, kind="stable")
    sslot, dslot, keep = sslot[order], dslot[order], keep[order]
    blk = dslot >> 7
    bstart = np.searchsorted(blk, np.arange(NBLK + 1))
    bcnt = np.diff(bstart)
    T = int((bcnt.max() + P - 1) // P)
    NT = NBPC * T

    # per-core [128, NT] metadata; edge i of block -> tile i//128, partition i%128
    srcS = np.zeros((NCORE, P, NT), np.int64)
    dloc = np.zeros((NCORE, P, NT), np.float32)
    kflag = np.zeros((NCORE, P, NT), bool)
    pad = np.ones((NCORE, P, NT), bool)
    for b in range(NBLK):
        c, lb = divmod(b, NBPC)
        m = bcnt[b]
        sl = slice(bstart[b], bstart[b + 1])
        fl_s = np.zeros(T * P, np.int64)
        fl_d = np.zeros(T * P, np.float32)
        fl_k = np.zeros(T * P, bool)
        fl_p = np.ones(T * P, bool)
        fl_s[:m] = sslot[sl]
        fl_d[:m] = (dslot[sl] & 127).astype(np.float32)
        fl_k[:m] = keep[sl]
        fl_p[:m] = False
        cols = slice(lb * T, (lb + 1) * T)
        srcS[c, :, cols] = fl_s.reshape(T, P).T
        dloc[c, :, cols] = fl_d.reshape(T, P).T
        kflag[c, :, cols] = fl_k.reshape(T, P).T
        pad[c, :, cols] = fl_p.reshape(T, P).T

    # ---- Launch A: h1x = X @ W1e ----
    A1 = np.zeros((F, H1 * C1), np.float32)
    A1d = np.zeros((F, H1 * C1), np.float32)
    W1 = np.asarray(W1, np.float32)
    As = np.zeros((H1 * C1, H1), np.float32)
    Ad = np.zeros((H1 * C1, H1), np.float32)
    for h in range(H1):
        As[h * C1:(h + 1) * C1, h] = np.asarray(att_src1)[h]
        Ad[h * C1:(h + 1) * C1, h] = np.asarray(att_dst1)[h]
    CP1 = C1 + 2
    M1 = H1 * CP1 + 2 * H1
    W1e = np.zeros((F, M1), np.float32)
    for h in range(H1):
        W1e[:, h * CP1:h * CP1 + C1] = W1[:, h * C1:(h + 1) * C1]
    W1e[:, H1 * CP1:H1 * CP1 + H1] = W1 @ As
    W1e[:, H1 * CP1 + H1:] = W1 @ Ad

    Xs = np.zeros((NSLOT, F), np.float32)
    Xs[slot_of_node] = X
    ncA = _build_dense(F, NLOC, M1)
    mapsA = [{"inT": np.ascontiguousarray(Xs[c * NLOC:(c + 1) * NLOC].T),
              "w": W1e} for c in range(NCORE)]
    resA = _run(ncA, mapsA, "A")
    h1x = np.concatenate([r["out"] for r in resA], axis=0)
    DBG['h1x'] = h1x; DBG['slot'] = slot_of_node

    h1row = h1x[:, :H1 * CP1].copy()
    for h in range(H1):
        h1row[:, h * CP1 + C1] = 1.0
        h1row[:, h * CP1 + C1 + 1] = 0.0
    a_src = h1x[:, H1 * CP1:H1 * CP1 + H1]
    a_dst = h1x[:, H1 * CP1 + H1:]

    # ---- Launch B: layer-1 aggregation ----
    b1v = np.asarray(b1, np.float32)
    hasb1 = bool(np.any(b1v))
    ncB = _build_edge(NBPC, T, H1, C1, relu_out=True, has_bias=hasb1)
    lb_of_nt = np.arange(NT) // T
    mapsB = []
    for c in range(NCORE):
        ss = srcS[c]
        dsl = ((c * NBPC + lb_of_nt) * P)[None, :] + dloc[c].astype(np.int64)
        asE3 = a_src[ss].astype(np.float32)           # [P, NT, H1]
        adE3 = a_dst[dsl].astype(np.float32)          # [P, NT, H1]
        asE3[pad[c]] = 0.0
        adE3[pad[c]] = -1e30
        asE3[kflag[c]] = 0.0
        adE3[kflag[c]] = 0.0
        asE = asE3.reshape(P, NT * H1)
        adE = adE3.reshape(P, NT * H1)
        hsE = h1row[ss.reshape(P, NBPC, T)].transpose(1, 0, 2, 3).reshape(
            NBPC * P, T * H1 * CP1)
        m = {"hsrcE": np.ascontiguousarray(hsE), "asrcE": asE, "adstE": adE,
             "dstloc": dloc[c].astype(np.float32)}
        if hasb1:
            m["biasbc"] = np.tile(b1v[None, :], (P, 1)).astype(np.float32)
        mapsB.append(m)
    resB = _run(ncB, mapsB, "B")
    g = np.concatenate([r["out"] for r in resB], axis=0)
    DBG['g'] = g

    # ---- Launch A2: h2x = g @ W2e ----
    W2 = np.asarray(W2, np.float32)
    CP2 = C2 + 2
    M2 = CP2 + 2
    W2e = np.zeros((H1 * C1, M2), np.float32)
    W2e[:, :C2] = W2
    W2e[:, CP2] = (W2 @ np.asarray(att_src2)[0]).astype(np.float32)
    W2e[:, CP2 + 1] = (W2 @ np.asarray(att_dst2)[0]).astype(np.float32)
    ncA2 = _build_dense(H1 * C1, NLOC, M2)
    mapsA2 = [{"inT": np.ascontiguousarray(g[c * NLOC:(c + 1) * NLOC].T),
               "w": W2e} for c in range(NCORE)]
    resA2 = _run(ncA2, mapsA2, "A2")
    h2x = np.concatenate([r["out"] for r in resA2], axis=0)
    h2row = h2x[:, :CP2].copy()
    h2row[:, C2] = 1.0
    h2row[:, C2 + 1] = 0.0
    a2s = h2x[:, CP2]
    a2d = h2x[:, CP2 + 1]

    # ---- Launch C: layer-2 aggregation ----
    b2v = np.asarray(b2, np.float32)
    hasb2 = bool(np.any(b2v))
    ncC = _build_edge(NBPC, T, 1, C2, relu_out=False, has_bias=hasb2)
    mapsC = []
    for c in range(NCORE):
        ss = srcS[c]
        dsl = ((c * NBPC + lb_of_nt) * P)[None, :] + dloc[c].astype(np.int64)
        asE = a2s[ss].astype(np.float32)
        adE = a2d[dsl].astype(np.float32)
        asE[pad[c]] = 0.0
        adE[pad[c]] = -1e30
        asE[kflag[c]] = 0.0
        adE[kflag[c]] = 0.0
        hsE = h2row[ss.reshape(P, NBPC, T)].transpose(1, 0, 2, 3).reshape(
            NBPC * P, T * CP2)
        m = {"hsrcE": np.ascontiguousarray(hsE), "asrcE": asE, "adstE": adE,
             "dstloc": dloc[c].astype(np.float32)}
        if hasb2:
            m["biasbc"] = np.tile(b2v[None, :], (P, 1)).astype(np.float32)
        mapsC.append(m)
    resC = _run(ncC, mapsC, "C")
    out_slots = np.concatenate([r["out"] for r in resC], axis=0)
    return np.ascontiguousarray(out_slots[slot_of_node]).astype(np.float32)

